# revision 49
# baseline (speedup 1.0000x reference)
"""Cosine-similarity multi-head attention on 8 Trainium2 NeuronCores.

Sharding: tensor-parallel over (batch, head-group). Core c (c = b*4 + hg)
computes heads [4*hg, 4*hg+4) of batch b for ALL 2048 query tokens, then a
partial output projection over its 256 inner features.  The host sums the 4
partial outputs per batch and adds b_out (the "all-reduce" of the hint, done
during the host-side gather).  No K/V duplication: each projection row is
computed exactly once across the machine.

Per-core layouts:
  - xt   [128, 8, 2048]  x[b]^T, feature-chunked (bf16)
  - qnT/knT [128, 2, 2048] Q^T/K^T: chunk m holds local heads 2m (parts 0:64)
    and 2m+1 (parts 64:128); normalized in place (bf16)
  - av   [128, 16, 4, 128] V token-major per (key-chunk, head); cols 0:64 are
    a ones block so each A@V matmul also accumulates softmax denominators
  - softmax: no max-subtraction (|logits| <= 10, exp safe in f32)
  - exp is split across engines: Act computes exact exp; the DVE computes a
    Schraudolph bitcast exp (scale+bias -> int16 convert = bf16 bits) for a
    share of the keys.  Softmax self-normalization + averaging over many keys
    keeps the ~3% sawtooth error far below the output tolerance.
  - norm factors are broadcast across partitions with small K=128 PE matmuls
    against a 0/1 head-selector matrix (hsq) -- no DRAM round trips.
  - out-projection runs as a final phase reusing the attention PSUM rings;
    partial outputs leave as bf16 (summed in f32 on the host).
"""

import numpy as np

B, N, DIM, H, DH = 2, 2048, 1024, 16, 64
INNER = H * DH
P = 128
KC = DIM // P        # 8 contraction chunks of the model dim
JC = N // P          # 16 key-token chunks of 128
QB = 4               # query blocks of 512
NQ = N // QB         # 512
HL = 4               # heads per core
M = 2                # feature chunks per core (4 heads * 64 = 256)
MAX_LOG_SCALE = float(np.log(1.0 / 0.01))

# Schraudolph fast-exp constants (int16 convert writes bf16 bit pattern)
EXP_A16 = (2.0 ** 23 / np.log(2.0)) / 65536.0
EXP_B16 = (127.0 * 2.0 ** 23 - 360777.0) / 65536.0 + 0.5

ACT_FULL_SET = (9,)  # key chunk where Act computes both heads (DVE-free)
WARMUP = 55          # junk N=128 matmuls bridging the input DMA so the PE
                     # clock gate stays released when real work arrives

_CACHE = {}


def _build():
    if "nc" in _CACHE:
        return _CACHE["nc"]
    import concourse.bass as bass
    import concourse.bacc as bacc
    import concourse.mybir as mybir
    import concourse.tile as tile

    f32 = mybir.dt.float32
    i16 = mybir.dt.int16
    bf16 = mybir.dt.bfloat16
    AF = mybir.ActivationFunctionType
    OP = mybir.AluOpType

    nc = bacc.Bacc("TRN2", target_bir_lowering=False)

    xTb = nc.declare_dram_parameter("xTb", [P, QB, KC, NQ], bf16,
                                    isOutput=False)
    wqb = nc.declare_dram_parameter("wqb", [P, KC, M, P], bf16, isOutput=False)
    wkb = nc.declare_dram_parameter("wkb", [P, KC, M, P], bf16, isOutput=False)
    wvb = nc.declare_dram_parameter("wvb", [P, KC, M * P], bf16, isOutput=False)
    wob = nc.declare_dram_parameter("wob", [P, M, KC, P], bf16, isOutput=False)
    hsq = nc.declare_dram_parameter("hsq", [P, M, P], bf16, isOutput=False)
    sclq2 = nc.declare_dram_parameter("sclq2", [P, M], f32, isOutput=False)
    outT = nc.declare_dram_parameter("outT", [KC, P, N], bf16, isOutput=True)

    with tile.TileContext(nc) as tc:
        with (
            tc.tile_pool(name="persist", bufs=1) as pp,
            tc.tile_pool(name="work", bufs=2) as pa,
            tc.tile_pool(name="pout", bufs=4) as pout,
            tc.tile_pool(name="ps", bufs=2, space="PSUM") as ps,
        ):
            # one tile (and one DMA) per 512-token block so the first
            # projection matmul only waits on its own block's transfer
            xts = [pp.tile([P, KC, NQ], bf16, tag=f"xt{i}", name=f"xt{i}")
                   for i in range(QB)]
            wu = pp.tile([P, P], bf16, tag="wu")
            qnT = pp.tile([P, M, N], bf16, tag="qnT")
            knT = pp.tile([P, M, N], bf16, tag="knT")
            av = pp.tile([P, JC, HL, DH], bf16, tag="av")
            ones_t = pp.tile([P, DH], bf16, tag="ones")
            onT = pp.tile([P, M, N], bf16, tag="onT")
            wq_sb = pp.tile([P, KC, M, P], bf16, tag="wq")
            wk_sb = pp.tile([P, KC, M, P], bf16, tag="wk")
            wv_sb = pp.tile([P, KC, M * P], bf16, tag="wv")
            wo_sb = pp.tile([P, M, KC, P], bf16, tag="wo")
            hsq_sb = pp.tile([P, M, P], bf16, tag="hsq")
            scl_sb = pp.tile([P, M], f32, tag="scl")
            zero_b = pp.tile([P, 1], f32, tag="zerob")

            # ---- input DMA: all five engine queues pull in parallel, each
            # queue ordered by consumption time (wk first, then x blocks in
            # order, then the later-needed weights) ----
            nc.sync.dma_start(out=hsq_sb[:], in_=hsq[:])
            nc.sync.dma_start(out=scl_sb[:], in_=sclq2[:])
            nc.scalar.dma_start(out=wk_sb[:, 0:4], in_=wkb[:, 0:4])
            nc.gpsimd.dma_start(out=wk_sb[:, 4:KC], in_=wkb[:, 4:KC])
            for i in range(QB):
                nc.scalar.dma_start(out=xts[i][:, 0:3, :], in_=xTb[:, i, 0:3])
                nc.gpsimd.dma_start(out=xts[i][:, 3:6, :],
                                    in_=xTb[:, i, 3:6])
                nc.sync.dma_start(out=xts[i][:, 6:KC, :],
                                  in_=xTb[:, i, 6:KC])
            nc.gpsimd.dma_start(out=wq_sb[:], in_=wqb[:])
            nc.scalar.dma_start(out=wv_sb[:], in_=wvb[:])
            nc.sync.dma_start(out=wo_sb[:], in_=wob[:])

            nc.vector.memset(zero_b[:], 0.0)
            nc.vector.memset(wu[:], 0.0)
            nc.vector.memset(ones_t[:], 1.0)

            # ---- PE warmup: junk matmuls during the initial DMA so the
            # clock gate is released before real work arrives; operand is a
            # memset tile so nothing waits on DMA ----
            warm = ps.tile([P, 2, NQ], f32, tag="sps", name="warm", bufs=3)
            for _ in range(WARMUP):
                nc.tensor.matmul(warm[:, 0, 0:P], wu[:], wu[:],
                                 start=True, stop=True)

            # ---------------- Phase A: K then Q projections + norms --------
            def factor_apply(nT, sqs_list, qb):
                qs = slice(qb * NQ, (qb + 1) * NQ)
                for m in range(M):
                    sqf = pa.tile([P, NQ], f32, tag="sqf")
                    nc.vector.reciprocal_approx_fast(
                        out=sqf[:], in_=sqs_list[m][:])
                    nc.gpsimd.tensor_mul(nT[:, m, qs], sqf[:], nT[:, m, qs])

            def emit_norm(pend_sq, q_scale):
                psq, pm, plist = pend_sq
                pn = ps.tile([P, NQ], f32, tag="dnB", name="pn", bufs=1)
                nc.tensor.matmul(pn[:], hsq_sb[:, pm, :], psq[:],
                                 start=True, stop=True)
                sqs = pa.tile([P, NQ], f32, tag="sqs", bufs=4)
                nc.scalar.activation(
                    sqs[:], pn[:], AF.Sqrt, bias=zero_b[:],
                    scale=scl_sb[:, pm : pm + 1] if q_scale else 1.0)
                plist.append(sqs)

            def proj_side(w_sb, nT, q_scale, after_first_factor=None):
                # norm matmul for each (qb, m) emitted one step late so the
                # PE never waits in-order on the Act square chain
                pend = None
                pend_sq = None
                for qb in range(QB):
                    qs = slice(qb * NQ, (qb + 1) * NQ)
                    sqs_list = []
                    for m in range(M):
                        pq = ps.tile([P, NQ], f32, name="pq",
                                     tag=("sps" if (qb * M + m) % 2 == 0
                                          else "avB"),
                                     bufs=(3 if (qb * M + m) % 2 == 0
                                           else 1))
                        for kc in range(KC):
                            nc.tensor.matmul(pq[:], w_sb[:, kc, m, :],
                                             xts[qb][:, kc, :],
                                             start=(kc == 0),
                                             stop=(kc == KC - 1))
                        nc.vector.tensor_copy(nT[:, m, qs], pq[:])
                        sq = pa.tile([P, NQ], bf16, tag="sq")
                        nc.scalar.activation(sq[:], pq[:], AF.Square,
                                             bias=zero_b[:])
                        if pend_sq is not None:
                            emit_norm(pend_sq, q_scale)
                        pend_sq = (sq, m, sqs_list)
                    if pend is not None:
                        factor_apply(nT, *pend)
                        if after_first_factor is not None:
                            after_first_factor()
                            after_first_factor = None
                    pend = (sqs_list, qb)
                emit_norm(pend_sq, q_scale)
                factor_apply(nT, *pend)

            proj_side(wk_sb, knT, False)  # K first (all keys needed first)
            proj_side(wq_sb, qnT, True)   # Q: temp folded into sqrt scale

            # ------------- Phase B: attention (+ V weave in sweep 0) -------
            def emit_v(jc):
                pv = ps.tile([P, M * P], f32, tag="sps", name="pv", bufs=3)
                xj = xts[jc // 4]
                js = slice((jc % 4) * P, (jc % 4) * P + P)
                for kc in range(KC):
                    nc.tensor.matmul(pv[:], xj[:, kc, js],
                                     wv_sb[:, kc, :],
                                     start=(kc == 0), stop=(kc == KC - 1))
                nc.vector.tensor_copy(
                    av[:, jc], pv[:].rearrange("p (h d) -> p h d", d=DH))

            pend = []
            norm_q = []   # deferred normalize ops, drained on DVE-free slots

            def flush_pend():
                if not pend:
                    return
                pet, pkc, pavB, pdnB, pqb, ppr = pend.pop(0)
                st, sp_ = (pkc == 0), (pkc == JC - 1)
                # heads col-tiled into one bank (h0 -> parts 0:64, h1 ->
                # 64:128, concurrent); denominators likewise in a second bank
                nc.tensor.matmul(pavB[0:DH, :], av[:, pkc, 2 * ppr],
                                 pet[:, 0, :], start=st, stop=sp_)
                nc.tensor.matmul(pavB[DH:P, :], av[:, pkc, 2 * ppr + 1],
                                 pet[:, 1, :], start=st, stop=sp_)
                nc.tensor.matmul(pdnB[0:DH, :], ones_t[:],
                                 pet[:, 0, :], start=st, stop=sp_)
                nc.tensor.matmul(pdnB[DH:P, :], ones_t[:],
                                 pet[:, 1, :], start=st, stop=sp_)
                if sp_:
                    # full-width reciprocal + one full-width multiply; the
                    # two DVE ops drain one per Act-full slot so they never
                    # delay a DVE exp
                    pqs = slice(pqb * NQ, (pqb + 1) * NQ)
                    stt = {}

                    def op_rec(pdnB=pdnB, stt=stt):
                        rec = pa.tile([P, NQ], f32, tag="dn", name="rec")
                        nc.vector.reciprocal_approx_fast(
                            out=rec[:], in_=pdnB[:])
                        stt["rec"] = rec

                    def op_mul(pavB=pavB, ppr=ppr, pqs=pqs, stt=stt):
                        nc.vector.tensor_mul(onT[:, ppr, pqs], pavB[:],
                                             stt["rec"][:])

                    norm_q.extend([op_rec, op_mul])

            for qb in range(QB):
                qs = slice(qb * NQ, (qb + 1) * NQ)
                for pr in range(M):       # head pair (2pr, 2pr+1)
                    avB = ps.tile([P, NQ], f32, tag="avB", name="avB", bufs=1)
                    dnB = ps.tile([P, NQ], f32, tag="dnB", name="dnB", bufs=1)
                    for kc in range(JC):
                        ks = slice(kc * P, (kc + 1) * P)
                        if qb == 0 and pr == 0:
                            emit_v(kc)    # weave V projection into sweep 0
                        sp = ps.tile([P, 2, NQ], f32, tag="sps", name="sp", bufs=3)
                        nc.tensor.matmul(sp[:, 0, :], knT[0:64, pr, ks],
                                         qnT[0:64, pr, qs],
                                         start=True, stop=True)
                        nc.tensor.matmul(sp[:, 1, :], knT[64:P, pr, ks],
                                         qnT[64:P, pr, qs],
                                         start=True, stop=True)
                        # A@V runs three kc behind (across sweep boundaries
                        # too) so the PE never waits in-order on exps still
                        # in flight on the Act/DVE queues
                        if len(pend) >= 3:
                            flush_pend()
                        # whole-tile exps alternate between the two engines:
                        # half the instruction count amortizes the ~120ns
                        # fixed cost per op on both queues
                        et = pa.tile([P, 2, NQ], bf16, tag="et", bufs=7)
                        if kc % 2 == 0:
                            nc.scalar.activation(et[:], sp[:], AF.Exp,
                                                 bias=zero_b[:])

                        else:
                            nc.vector.tensor_scalar(
                                out=et[:].bitcast(i16),
                                in0=sp[:],
                                scalar1=EXP_A16, scalar2=EXP_B16,
                                op0=OP.mult, op1=OP.add)
                        pend.append((et, kc, avB, dnB, qb, pr))
                    # drain fully at sweep end: the single avB/dnB banks
                    # must be normalized (chain runs in the next sweep's
                    # first DVE-free slot) before their next start=True
                    while pend:
                        flush_pend()
                    while norm_q:
                        norm_q.pop(0)()

            # ------------- Phase C: output projection ----------------------
            # batch 4 feature chunks per output DMA: trigger instructions
            # cost ~0.9us each on the issuing queue, so fewer is faster
            for qb in range(QB):
                qs = slice(qb * NQ, (qb + 1) * NQ)
                for half in range(2):
                    ob = pout.tile([P, 4, NQ], bf16, tag="ot", name="ob",
                                   bufs=3)
                    for j in range(4):
                        mo = half * 4 + j
                        idx = qb * KC + mo
                        cp = ps.tile([P, NQ], f32,
                                     tag=("avB" if idx % 2 == 0 else "dnB"),
                                     name="cp", bufs=1)
                        for g in range(M):
                            nc.tensor.matmul(cp[:], wo_sb[:, g, mo, :],
                                             onT[:, g, qs],
                                             start=(g == 0),
                                             stop=(g == M - 1))
                        if idx % 2 == 0:
                            nc.vector.tensor_copy(ob[:, j, :], cp[:])
                        else:
                            nc.scalar.copy(ob[:, j, :], cp[:])
                    eng = nc.sync if (qb * 2 + half) % 2 == 0 else nc.gpsimd
                    if qb == QB - 1:
                        # final block: per-chunk DMAs overlap the evac tail
                        for j in range(4):
                            mo = half * 4 + j
                            eng = nc.sync if j % 2 == 0 else nc.gpsimd
                            eng.dma_start(out=outT[mo, :, qs],
                                          in_=ob[:, j, :])
                    else:
                        eng.dma_start(
                            out=outT[half * 4 : half * 4 + 4, :, qs]
                            .rearrange("k p n -> p k n"),
                            in_=ob[:])

    nc.compile()
    _CACHE["nc"] = nc
    return nc


def run(inputs, trace=False):
    import ml_dtypes
    from concourse.bass_utils import run_bass_kernel_spmd

    x = np.asarray(inputs["x"], np.float32)
    w_qkv = np.asarray(inputs["w_qkv"], np.float32)
    w_out = np.asarray(inputs["w_out"], np.float32)
    b_out = np.asarray(inputs["b_out"], np.float32)
    logit_scale = np.asarray(inputs["logit_scale"], np.float32)

    nc = _build()
    bf = ml_dtypes.bfloat16

    scl = np.exp(np.minimum(logit_scale.reshape(H), MAX_LOG_SCALE))

    # [P, QB, KC, NQ]: partition-major, then 512-token block, then dim chunk
    xTb = [np.ascontiguousarray(
        x[b].T.reshape(KC, P, QB, NQ).transpose(1, 2, 0, 3)).astype(bf)
        for b in range(B)]

    in_maps = []
    for c in range(8):
        b, hg = c // 4, c % 4
        cs = slice(hg * 256, (hg + 1) * 256)
        wq = np.ascontiguousarray(
            w_qkv[:, 0:INNER][:, cs].reshape(KC, P, M, P)
            .transpose(1, 0, 2, 3)).astype(bf)
        wk = np.ascontiguousarray(
            w_qkv[:, INNER:2 * INNER][:, cs].reshape(KC, P, M, P)
            .transpose(1, 0, 2, 3)).astype(bf)
        wv = np.ascontiguousarray(
            w_qkv[:, 2 * INNER:3 * INNER][:, cs].reshape(KC, P, M * P)
            .transpose(1, 0, 2)).astype(bf)
        wo = np.ascontiguousarray(
            w_out[cs, :].reshape(M, P, KC, P).transpose(1, 0, 2, 3)).astype(bf)
        hs = np.zeros((P, M, P), bf)
        hs[0:64, :, 0:64] = 1.0
        hs[64:P, :, 64:P] = 1.0
        sc2 = np.empty((P, M), np.float32)
        for m in range(M):
            sc2[0:64, m] = scl[4 * hg + 2 * m] ** -2.0
            sc2[64:P, m] = scl[4 * hg + 2 * m + 1] ** -2.0
        in_maps.append({
            "xTb": xTb[b], "wqb": wq, "wkb": wk, "wvb": wv, "wob": wo,
            "hsq": hs, "sclq2": sc2,
        })

    res = run_bass_kernel_spmd(nc, in_maps, list(range(8)), trace=trace)

    out = np.empty((B, N, DIM), np.float32)
    for b in range(B):
        acc = res.results[4 * b]["outT"].reshape(DIM, N).astype(np.float32)
        for hg in range(1, 4):
            acc = acc + res.results[4 * b + hg]["outT"].reshape(DIM, N)\
                .astype(np.float32)
        out[b] = acc.T + b_out
    return out, res


def kernel(**inputs):
    out, _ = run(inputs, trace=False)
    return out


# revision 50
# speedup vs baseline: 1.0139x; 1.0139x over previous
"""Cosine-similarity multi-head attention on 8 Trainium2 NeuronCores.

Sharding: tensor-parallel over (batch, head-group). Core c (c = b*4 + hg)
computes heads [4*hg, 4*hg+4) of batch b for ALL 2048 query tokens, then a
partial output projection over its 256 inner features.  The host sums the 4
partial outputs per batch and adds b_out (the "all-reduce" of the hint, done
during the host-side gather).  No K/V duplication: each projection row is
computed exactly once across the machine.

Per-core layouts:
  - xt   [128, 8, 2048]  x[b]^T, feature-chunked (bf16)
  - qnT/knT [128, 2, 2048] Q^T/K^T: chunk m holds local heads 2m (parts 0:64)
    and 2m+1 (parts 64:128); normalized in place (bf16)
  - av   [128, 16, 4, 128] V token-major per (key-chunk, head); cols 0:64 are
    a ones block so each A@V matmul also accumulates softmax denominators
  - softmax: no max-subtraction (|logits| <= 10, exp safe in f32)
  - exp is split across engines: Act computes exact exp; the DVE computes a
    Schraudolph bitcast exp (scale+bias -> int16 convert = bf16 bits) for a
    share of the keys.  Softmax self-normalization + averaging over many keys
    keeps the ~3% sawtooth error far below the output tolerance.
  - norm factors are broadcast across partitions with small K=128 PE matmuls
    against a 0/1 head-selector matrix (hsq) -- no DRAM round trips.
  - out-projection runs as a final phase reusing the attention PSUM rings;
    partial outputs leave as bf16 (summed in f32 on the host).
"""

import numpy as np

B, N, DIM, H, DH = 2, 2048, 1024, 16, 64
INNER = H * DH
P = 128
KC = DIM // P        # 8 contraction chunks of the model dim
JC = N // P          # 16 key-token chunks of 128
QB = 4               # query blocks of 512
NQ = N // QB         # 512
HL = 4               # heads per core
M = 2                # feature chunks per core (4 heads * 64 = 256)
MAX_LOG_SCALE = float(np.log(1.0 / 0.01))

# Schraudolph fast-exp constants (int16 convert writes bf16 bit pattern)
EXP_A16 = (2.0 ** 23 / np.log(2.0)) / 65536.0
EXP_B16 = (127.0 * 2.0 ** 23 - 360777.0) / 65536.0 + 0.5

ACT_FULL_SET = (9,)  # key chunk where Act computes both heads (DVE-free)
WARMUP = 55          # junk N=128 matmuls bridging the input DMA so the PE
                     # clock gate stays released when real work arrives

_CACHE = {}


def _build():
    if "nc" in _CACHE:
        return _CACHE["nc"]
    import concourse.bass as bass
    import concourse.bacc as bacc
    import concourse.mybir as mybir
    import concourse.tile as tile

    f32 = mybir.dt.float32
    i16 = mybir.dt.int16
    bf16 = mybir.dt.bfloat16
    AF = mybir.ActivationFunctionType
    OP = mybir.AluOpType

    nc = bacc.Bacc("TRN2", target_bir_lowering=False)

    xTb = nc.declare_dram_parameter("xTb", [P, QB, KC, NQ], bf16,
                                    isOutput=False)
    wqb = nc.declare_dram_parameter("wqb", [P, KC, M, P], bf16, isOutput=False)
    wkb = nc.declare_dram_parameter("wkb", [P, KC, M, P], bf16, isOutput=False)
    wvb = nc.declare_dram_parameter("wvb", [P, KC, M * P], bf16, isOutput=False)
    wob = nc.declare_dram_parameter("wob", [P, M, KC, P], bf16, isOutput=False)
    hsq = nc.declare_dram_parameter("hsq", [P, M, P], bf16, isOutput=False)
    sclq2 = nc.declare_dram_parameter("sclq2", [P, M], f32, isOutput=False)
    outT = nc.declare_dram_parameter("outT", [KC, P, N], bf16, isOutput=True)

    with tile.TileContext(nc) as tc:
        with (
            tc.tile_pool(name="persist", bufs=1) as pp,
            tc.tile_pool(name="work", bufs=2) as pa,
            tc.tile_pool(name="pout", bufs=4) as pout,
            tc.tile_pool(name="ps", bufs=2, space="PSUM") as ps,
        ):
            # one tile (and one DMA) per 512-token block so the first
            # projection matmul only waits on its own block's transfer
            xts = [pp.tile([P, KC, NQ], bf16, tag=f"xt{i}", name=f"xt{i}")
                   for i in range(QB)]
            wu = pp.tile([P, P], bf16, tag="wu")
            qnT = pp.tile([P, M, N], bf16, tag="qnT")
            knT = pp.tile([P, M, N], bf16, tag="knT")
            av = pp.tile([P, JC, HL, DH], bf16, tag="av")
            ones_t = pp.tile([P, DH], bf16, tag="ones")
            onT = pp.tile([P, M, N], bf16, tag="onT")
            wq_sb = pp.tile([P, KC, M, P], bf16, tag="wq")
            wk_sb = pp.tile([P, KC, M, P], bf16, tag="wk")
            wv_sb = pp.tile([P, KC, M * P], bf16, tag="wv")
            wo_sb = pp.tile([P, M, KC, P], bf16, tag="wo")
            hsq_sb = pp.tile([P, M, P], bf16, tag="hsq")
            scl_sb = pp.tile([P, M], f32, tag="scl")
            zero_b = pp.tile([P, 1], f32, tag="zerob")

            # ---- input DMA: all five engine queues pull in parallel, each
            # queue ordered by consumption time (wk first, then x blocks in
            # order, then the later-needed weights) ----
            nc.sync.dma_start(out=hsq_sb[:], in_=hsq[:])
            nc.sync.dma_start(out=scl_sb[:], in_=sclq2[:])
            nc.scalar.dma_start(out=wk_sb[:, 0:4], in_=wkb[:, 0:4])
            nc.gpsimd.dma_start(out=wk_sb[:, 4:KC], in_=wkb[:, 4:KC])
            for i in range(QB):
                nc.scalar.dma_start(out=xts[i][:, 0:3, :], in_=xTb[:, i, 0:3])
                nc.gpsimd.dma_start(out=xts[i][:, 3:6, :],
                                    in_=xTb[:, i, 3:6])
                nc.sync.dma_start(out=xts[i][:, 6:KC, :],
                                  in_=xTb[:, i, 6:KC])
            nc.gpsimd.dma_start(out=wq_sb[:], in_=wqb[:])
            nc.scalar.dma_start(out=wv_sb[:], in_=wvb[:])
            nc.sync.dma_start(out=wo_sb[:], in_=wob[:])

            nc.vector.memset(zero_b[:], 0.0)
            nc.vector.memset(wu[:], 0.0)
            nc.vector.memset(ones_t[:], 1.0)

            # ---- PE warmup: junk matmuls during the initial DMA so the
            # clock gate is released before real work arrives; operand is a
            # memset tile so nothing waits on DMA ----
            warm = ps.tile([P, 2, NQ], f32, tag="sps", name="warm")
            for _ in range(WARMUP):
                nc.tensor.matmul(warm[:, 0, 0:P], wu[:], wu[:],
                                 start=True, stop=True)

            # ---------------- Phase A: K then Q projections + norms --------
            def factor_apply(nT, sqs_list, qb):
                qs = slice(qb * NQ, (qb + 1) * NQ)
                for m in range(M):
                    sqf = pa.tile([P, NQ], f32, tag="sqf")
                    nc.vector.reciprocal_approx_fast(
                        out=sqf[:], in_=sqs_list[m][:])
                    nc.gpsimd.tensor_mul(nT[:, m, qs], sqf[:], nT[:, m, qs])

            def emit_norm(pend_sq, q_scale):
                psq, pm, plist = pend_sq
                pn = ps.tile([P, NQ], f32, tag="dnB", name="pn")
                nc.tensor.matmul(pn[:], hsq_sb[:, pm, :], psq[:],
                                 start=True, stop=True)
                sqs = pa.tile([P, NQ], f32, tag="sqs", bufs=4)
                nc.scalar.activation(
                    sqs[:], pn[:], AF.Sqrt, bias=zero_b[:],
                    scale=scl_sb[:, pm : pm + 1] if q_scale else 1.0)
                plist.append(sqs)

            def proj_side(w_sb, nT, q_scale, after_first_factor=None):
                # norm matmul for each (qb, m) emitted one step late so the
                # PE never waits in-order on the Act square chain
                pend = None
                pend_sq = None
                for qb in range(QB):
                    qs = slice(qb * NQ, (qb + 1) * NQ)
                    sqs_list = []
                    for m in range(M):
                        pq = ps.tile([P, NQ], f32, name="pq",
                                     tag=("sps" if (qb * M + m) % 2 == 0
                                          else "avB"))
                        for kc in range(KC):
                            nc.tensor.matmul(pq[:], w_sb[:, kc, m, :],
                                             xts[qb][:, kc, :],
                                             start=(kc == 0),
                                             stop=(kc == KC - 1))
                        nc.vector.tensor_copy(nT[:, m, qs], pq[:])
                        sq = pa.tile([P, NQ], bf16, tag="sq")
                        nc.scalar.activation(sq[:], pq[:], AF.Square,
                                             bias=zero_b[:])
                        if pend_sq is not None:
                            emit_norm(pend_sq, q_scale)
                        pend_sq = (sq, m, sqs_list)
                    if pend is not None:
                        factor_apply(nT, *pend)
                        if after_first_factor is not None:
                            after_first_factor()
                            after_first_factor = None
                    pend = (sqs_list, qb)
                emit_norm(pend_sq, q_scale)
                factor_apply(nT, *pend)

            proj_side(wk_sb, knT, False)  # K first (all keys needed first)
            proj_side(wq_sb, qnT, True)   # Q: temp folded into sqrt scale

            # ------------- Phase B: attention (+ V weave in sweep 0) -------
            def emit_v(jc):
                pv = ps.tile([P, M * P], f32, tag="sps", name="pv")
                xj = xts[jc // 4]
                js = slice((jc % 4) * P, (jc % 4) * P + P)
                for kc in range(KC):
                    nc.tensor.matmul(pv[:], xj[:, kc, js],
                                     wv_sb[:, kc, :],
                                     start=(kc == 0), stop=(kc == KC - 1))
                nc.vector.tensor_copy(
                    av[:, jc], pv[:].rearrange("p (h d) -> p h d", d=DH))

            pend = []
            norm_q = []   # deferred normalize ops, drained on DVE-free slots

            def flush_pend():
                if not pend:
                    return
                pet, pkc, pavB, pdnB, pqb, ppr = pend.pop(0)
                st, sp_ = (pkc == 0), (pkc == JC - 1)
                # heads col-tiled into one bank (h0 -> parts 0:64, h1 ->
                # 64:128, concurrent); denominators likewise in a second bank
                nc.tensor.matmul(pavB[0:DH, :], av[:, pkc, 2 * ppr],
                                 pet[:, 0, :], start=st, stop=sp_)
                nc.tensor.matmul(pavB[DH:P, :], av[:, pkc, 2 * ppr + 1],
                                 pet[:, 1, :], start=st, stop=sp_)
                nc.tensor.matmul(pdnB[0:DH, :], ones_t[:],
                                 pet[:, 0, :], start=st, stop=sp_)
                nc.tensor.matmul(pdnB[DH:P, :], ones_t[:],
                                 pet[:, 1, :], start=st, stop=sp_)
                if sp_:
                    # full-width reciprocal + one full-width multiply; the
                    # two DVE ops drain one per Act-full slot so they never
                    # delay a DVE exp
                    pqs = slice(pqb * NQ, (pqb + 1) * NQ)
                    stt = {}

                    def op_rec(pdnB=pdnB, stt=stt):
                        rec = pa.tile([P, NQ], f32, tag="dn", name="rec")
                        nc.vector.reciprocal_approx_fast(
                            out=rec[:], in_=pdnB[:])
                        stt["rec"] = rec

                    def op_mul(pavB=pavB, ppr=ppr, pqs=pqs, stt=stt):
                        nc.vector.tensor_mul(onT[:, ppr, pqs], pavB[:],
                                             stt["rec"][:])

                    norm_q.extend([op_rec, op_mul])

            for qb in range(QB):
                qs = slice(qb * NQ, (qb + 1) * NQ)
                for pr in range(M):       # head pair (2pr, 2pr+1)
                    avB = ps.tile([P, NQ], f32, tag="avB", name="avB")
                    dnB = ps.tile([P, NQ], f32, tag="dnB", name="dnB")
                    for kc in range(JC):
                        ks = slice(kc * P, (kc + 1) * P)
                        if qb == 0 and pr == 0:
                            emit_v(kc)    # weave V projection into sweep 0
                        sp = ps.tile([P, 2, NQ], f32, tag="sps", name="sp")
                        nc.tensor.matmul(sp[:, 0, :], knT[0:64, pr, ks],
                                         qnT[0:64, pr, qs],
                                         start=True, stop=True)
                        nc.tensor.matmul(sp[:, 1, :], knT[64:P, pr, ks],
                                         qnT[64:P, pr, qs],
                                         start=True, stop=True)
                        # A@V runs three kc behind (across sweep boundaries
                        # too) so the PE never waits in-order on exps still
                        # in flight on the Act/DVE queues
                        if len(pend) >= 4:
                            flush_pend()
                        # whole-tile exps alternate between the two engines:
                        # half the instruction count amortizes the ~120ns
                        # fixed cost per op on both queues
                        et = pa.tile([P, 2, NQ], bf16, tag="et", bufs=7)
                        if kc % 2 == 0:
                            nc.scalar.activation(et[:], sp[:], AF.Exp,
                                                 bias=zero_b[:])
                            if kc >= 4 and norm_q:
                                norm_q.pop(0)()
                        else:
                            nc.vector.tensor_scalar(
                                out=et[:].bitcast(i16),
                                in0=sp[:],
                                scalar1=EXP_A16, scalar2=EXP_B16,
                                op0=OP.mult, op1=OP.add)
                        pend.append((et, kc, avB, dnB, qb, pr))
            while pend:
                flush_pend()
            while norm_q:
                norm_q.pop(0)()

            # ------------- Phase C: output projection ----------------------
            # batch 4 feature chunks per output DMA: trigger instructions
            # cost ~0.9us each on the issuing queue, so fewer is faster
            for qb in range(QB):
                qs = slice(qb * NQ, (qb + 1) * NQ)
                for half in range(2):
                    ob = pout.tile([P, 4, NQ], bf16, tag="ot", name="ob",
                                   bufs=3)
                    for j in range(4):
                        mo = half * 4 + j
                        idx = qb * KC + mo
                        cp = ps.tile([P, NQ], f32,
                                     tag=("avB" if idx % 2 == 0 else "dnB"),
                                     name="cp")
                        for g in range(M):
                            nc.tensor.matmul(cp[:], wo_sb[:, g, mo, :],
                                             onT[:, g, qs],
                                             start=(g == 0),
                                             stop=(g == M - 1))
                        if idx % 2 == 0:
                            nc.vector.tensor_copy(ob[:, j, :], cp[:])
                        else:
                            nc.scalar.copy(ob[:, j, :], cp[:])
                    eng = nc.sync if (qb * 2 + half) % 2 == 0 else nc.gpsimd
                    if qb == QB - 1:
                        # final block: per-chunk DMAs overlap the evac tail
                        for j in range(4):
                            mo = half * 4 + j
                            eng = nc.sync if j % 2 == 0 else nc.gpsimd
                            eng.dma_start(out=outT[mo, :, qs],
                                          in_=ob[:, j, :])
                    else:
                        eng.dma_start(
                            out=outT[half * 4 : half * 4 + 4, :, qs]
                            .rearrange("k p n -> p k n"),
                            in_=ob[:])

    nc.compile()
    _CACHE["nc"] = nc
    return nc


def run(inputs, trace=False):
    import ml_dtypes
    from concourse.bass_utils import run_bass_kernel_spmd

    x = np.asarray(inputs["x"], np.float32)
    w_qkv = np.asarray(inputs["w_qkv"], np.float32)
    w_out = np.asarray(inputs["w_out"], np.float32)
    b_out = np.asarray(inputs["b_out"], np.float32)
    logit_scale = np.asarray(inputs["logit_scale"], np.float32)

    nc = _build()
    bf = ml_dtypes.bfloat16

    scl = np.exp(np.minimum(logit_scale.reshape(H), MAX_LOG_SCALE))

    # [P, QB, KC, NQ]: partition-major, then 512-token block, then dim chunk
    xTb = [np.ascontiguousarray(
        x[b].T.reshape(KC, P, QB, NQ).transpose(1, 2, 0, 3)).astype(bf)
        for b in range(B)]

    in_maps = []
    for c in range(8):
        b, hg = c // 4, c % 4
        cs = slice(hg * 256, (hg + 1) * 256)
        wq = np.ascontiguousarray(
            w_qkv[:, 0:INNER][:, cs].reshape(KC, P, M, P)
            .transpose(1, 0, 2, 3)).astype(bf)
        wk = np.ascontiguousarray(
            w_qkv[:, INNER:2 * INNER][:, cs].reshape(KC, P, M, P)
            .transpose(1, 0, 2, 3)).astype(bf)
        wv = np.ascontiguousarray(
            w_qkv[:, 2 * INNER:3 * INNER][:, cs].reshape(KC, P, M * P)
            .transpose(1, 0, 2)).astype(bf)
        wo = np.ascontiguousarray(
            w_out[cs, :].reshape(M, P, KC, P).transpose(1, 0, 2, 3)).astype(bf)
        hs = np.zeros((P, M, P), bf)
        hs[0:64, :, 0:64] = 1.0
        hs[64:P, :, 64:P] = 1.0
        sc2 = np.empty((P, M), np.float32)
        for m in range(M):
            sc2[0:64, m] = scl[4 * hg + 2 * m] ** -2.0
            sc2[64:P, m] = scl[4 * hg + 2 * m + 1] ** -2.0
        in_maps.append({
            "xTb": xTb[b], "wqb": wq, "wkb": wk, "wvb": wv, "wob": wo,
            "hsq": hs, "sclq2": sc2,
        })

    res = run_bass_kernel_spmd(nc, in_maps, list(range(8)), trace=trace)

    out = np.empty((B, N, DIM), np.float32)
    for b in range(B):
        acc = res.results[4 * b]["outT"].reshape(DIM, N).astype(np.float32)
        for hg in range(1, 4):
            acc = acc + res.results[4 * b + hg]["outT"].reshape(DIM, N)\
                .astype(np.float32)
        out[b] = acc.T + b_out
    return out, res


def kernel(**inputs):
    out, _ = run(inputs, trace=False)
    return out
